# revision 3
# baseline (speedup 1.0000x reference)
"""ColorNorm Trainium2 kernel.

Problem: per-sample 3x3 color-matching solve over N=1024*1024 pixels.
  A = src[b] (3,N), B = dst[b] (3,N)
  AAt = Ac@Ac.T + 1e-3 I ; BAt = Bc@Ac.T ; x = BAt@inv(AAt)
  out[b] = x@Ac + Bmean
Sharding: data-parallel over batch (16 samples -> 8 cores x 2 samples).

Per-core pipeline (all on one NeuronCore, fp32 end-to-end):
  pass1: stream A,B; DVE tensor_tensor_reduce computes the 15 product-sums
         (6 AA + 9 BA) fused with per-partition reduction; ScalarE
         activation(Copy, accum_out) computes raw channel sums.
         A single ones-matmul on PE does the final cross-partition reduce.
  solve: 3x3 inverse via adjugate (tiny DVE ops on partition 0).
  pass2: out_i = sum_j x_ij*A_j + d_i via PE matmuls with diag(x_ij)
         stationary weights accumulating in PSUM; ScalarE evicts with the
         +d_i bias fused; 1MiB store DMAs.
"""

import sys

for _p in ("/opt/trn_rl_repo", "/opt/pypackages"):
    if _p not in sys.path:
        sys.path.append(_p)

from contextlib import ExitStack

import numpy as np

import concourse.bacc as bacc
import concourse.bass as bass
import concourse.tile as tile
from concourse import masks, mybir
from concourse._compat import with_exitstack

# ---- hardcoded problem geometry (per core) ----
B_CORE = 2          # samples per core
C = 3               # channels
H = W = 1024
N = H * W           # 1048576 pixels per channel
P = 128             # SBUF partitions
F = N // P          # 8192 free elems per partition per channel
Q = 2048            # quarter-chunk free size (1 MiB DMAs)
NQ = F // Q         # 4 quarters
MM = 512            # matmul free-dim chunk (one PSUM bank)
NCORES = 8
RIDGE = 1e-3

F32 = mybir.dt.float32
ALU = mybir.AluOpType
ACTF = mybir.ActivationFunctionType

# 6 unique AA pairs; symmetric index map
A_PAIRS = [(0, 0), (0, 1), (0, 2), (1, 1), (1, 2), (2, 2)]
SYM = {(0, 0): 0, (0, 1): 1, (0, 2): 2, (1, 1): 3, (1, 2): 4, (2, 2): 5}


def _rd(ap, dims):
    """Rebuild an AP keeping its partition dim, replacing free dims."""
    return bass.AP(ap.tensor, ap.offset, [ap.ap[0]] + dims)


@with_exitstack
def _colornorm(ctx: ExitStack, tc: "tile.TileContext", src, dst, out):
    nc = tc.nc
    srcv = src.rearrange("b c (p q) w -> b c p (q w)", p=P)  # [2,3,128,8192]
    dstv = dst.rearrange("b c (p q) w -> b c p (q w)", p=P)
    outv = out.rearrange("b c (p q) w -> b c p (q w)", p=P)

    singles = ctx.enter_context(tc.tile_pool(name="singles", bufs=1))
    a_pool = ctx.enter_context(tc.tile_pool(name="a_pool", bufs=14))
    b_pool = ctx.enter_context(tc.tile_pool(name="b_pool", bufs=2))
    scr_pool = ctx.enter_context(tc.tile_pool(name="scr", bufs=1))
    ascr_pool = ctx.enter_context(tc.tile_pool(name="ascr", bufs=1))
    acc_pool = ctx.enter_context(tc.tile_pool(name="accs", bufs=2))
    solve_pool = ctx.enter_context(tc.tile_pool(name="solve", bufs=2))
    dg_pool = ctx.enter_context(tc.tile_pool(name="dg", bufs=1))
    stage_pool = ctx.enter_context(tc.tile_pool(name="stage", bufs=3))
    ps_stat = ctx.enter_context(tc.tile_pool(name="ps_stat", bufs=2, space="PSUM"))
    ps_out = ctx.enter_context(tc.tile_pool(name="ps_out", bufs=6, space="PSUM"))

    ones = singles.tile([P, 1], F32)
    nc.vector.memset(ones, 1.0)
    eye = singles.tile([P, P], F32)
    masks.make_identity(nc, eye[:])

    for s in range(B_CORE):
        # ---------------- pass 1: load + statistics ----------------
        a_t = [[a_pool.tile([P, Q], F32, tag="aq", name="aq") for _ in range(NQ)]
               for _ in range(C)]
        b_t = [[None] * NQ for _ in range(C)]
        for c in range(C):
            for q in range(NQ):
                nc.sync.dma_start(out=a_t[c][q][:],
                                  in_=srcv[s, c][:, q * Q:(q + 1) * Q])
        for c in range(C):
            for q in range(NQ):
                b_t[c][q] = b_pool.tile([P, Q], F32, tag="bq", name="bq")
                nc.sync.dma_start(out=b_t[c][q][:],
                                  in_=dstv[s, c][:, q * Q:(q + 1) * Q])

        # per-partition accumulators: 15 pairs x 4 quarters product partials,
        # 24 raw partials
        accP = acc_pool.tile([P, 64], F32, tag="accP", name="accP")
        acc_raw = acc_pool.tile([P, 24], F32, tag="acc_raw", name="acc_raw")

        # raw A sums on ScalarE (col = c*NQ+q)
        for c in range(C):
            for q in range(NQ):
                ascr = ascr_pool.tile([P, Q], F32, tag="ascr", name="ascr")
                nc.scalar.activation(
                    out=ascr[:], in_=a_t[c][q][:], func=ACTF.Copy,
                    accum_out=acc_raw[:, c * NQ + q: c * NQ + q + 1])

        def ttr(k, q, x_ap, y_ap):
            """fused product+reduce into accumulator column (k, q)"""
            scr = scr_pool.tile([P, Q], F32, tag="scr", name="scr")
            col = k * NQ + q
            nc.vector.scalar_tensor_tensor(
                out=scr[:], in0=x_ap, scalar=1.0, in1=y_ap,
                op0=ALU.mult, op1=ALU.mult,
                accum_out=accP[:, col:col + 1])

        # AA products (cols 0..5)
        for q in range(NQ):
            for k, (i, j) in enumerate(A_PAIRS):
                ttr(k, q, a_t[i][q][:], a_t[j][q][:])
        # BA products (cols 6..14) + raw B sums (cols 12..23 of acc_raw)
        for c in range(C):
            for q in range(NQ):
                ascr = ascr_pool.tile([P, Q], F32, tag="ascr", name="ascr")
                nc.scalar.activation(
                    out=ascr[:], in_=b_t[c][q][:], func=ACTF.Copy,
                    accum_out=acc_raw[:, 12 + c * NQ + q: 13 + c * NQ + q])
                for j in range(C):
                    ttr(6 + c * 3 + j, q, b_t[c][q][:], a_t[j][q][:])

        # cross-partition reduce on PE: ones.T @ acc -> [1, n]
        pst = ps_stat.tile([1, 96], F32, tag="pst", name="pst")
        nc.tensor.matmul(pst[0:1, 0:60], ones[:], accP[:, 0:60],
                         start=True, stop=True)
        nc.tensor.matmul(pst[0:1, 64:88], ones[:], acc_raw[:],
                         start=True, stop=True)
        stats = solve_pool.tile([1, 96], F32, tag="stats", name="stats")
        nc.vector.tensor_copy(out=stats[:], in_=pst[0:1, 0:96])
        # collapse the 4 quarter partials of each product pair
        prod15 = solve_pool.tile([1, 15], F32, tag="prod15", name="prod15")
        nc.vector.reduce_sum(out=prod15[:], axis=mybir.AxisListType.X,
                             in_=stats[0:1, 0:60].rearrange(
                                 "p (k q) -> p k q", q=NQ))

        # ---------------- 3x3 solve on partition 0 ----------------
        sumA = solve_pool.tile([1, 3], F32, tag="sumA", name="sumA")
        sumB = solve_pool.tile([1, 3], F32, tag="sumB", name="sumB")
        nc.vector.reduce_sum(out=sumA[:], axis=mybir.AxisListType.X,
                             in_=stats[0:1, 64:76].rearrange(
                                 "p (c q) -> p c q", q=NQ))
        nc.vector.reduce_sum(out=sumB[:], axis=mybir.AxisListType.X,
                             in_=stats[0:1, 76:88].rearrange(
                                 "p (c q) -> p c q", q=NQ))
        Am = solve_pool.tile([1, 3], F32, tag="Am", name="Am")
        Bm = solve_pool.tile([1, 3], F32, tag="Bm", name="Bm")
        nc.vector.tensor_scalar_mul(out=Am[:], in0=sumA[:], scalar1=1.0 / N)
        nc.vector.tensor_scalar_mul(out=Bm[:], in0=sumB[:], scalar1=1.0 / N)

        AA9 = solve_pool.tile([1, 9], F32, tag="AA9", name="AA9")
        for i in range(C):
            for j in range(C):
                k = SYM[(min(i, j), max(i, j))]
                nc.vector.tensor_copy(out=AA9[0:1, 3 * i + j: 3 * i + j + 1],
                                      in_=prod15[0:1, k:k + 1])
        BA9 = solve_pool.tile([1, 9], F32, tag="BA9", name="BA9")
        nc.vector.tensor_copy(out=BA9[:], in_=prod15[0:1, 6:15])

        # centered covariances: AAc = AA - N*Am Am^T (+ridge); BAc = BA - N*Bm Am^T
        outer = solve_pool.tile([1, 9], F32, tag="outer", name="outer")
        o3x3 = outer[0:1, :].rearrange("p (i j) -> p i j", j=3)
        nc.vector.tensor_mul(out=o3x3, in0=_rd(Am[0:1, 0:1], [[1, 3], [0, 3]]),
                             in1=_rd(Am[0:1, 0:1], [[0, 3], [1, 3]]))
        AAc = solve_pool.tile([1, 9], F32, tag="AAc", name="AAc")
        nc.vector.scalar_tensor_tensor(out=AAc[:], in0=outer[:],
                                       scalar=-float(N), in1=AA9[:],
                                       op0=ALU.mult, op1=ALU.add)
        dg_ap = _rd(AAc[0:1, 0:1], [[4, 3]])
        nc.vector.tensor_scalar_add(out=dg_ap, in0=dg_ap, scalar1=RIDGE)
        nc.vector.tensor_mul(out=o3x3, in0=_rd(Bm[0:1, 0:1], [[1, 3], [0, 3]]),
                             in1=_rd(Am[0:1, 0:1], [[0, 3], [1, 3]]))
        BAc = solve_pool.tile([1, 9], F32, tag="BAc", name="BAc")
        nc.vector.scalar_tensor_tensor(out=BAc[:], in0=outer[:],
                                       scalar=-float(N), in1=BA9[:],
                                       op0=ALU.mult, op1=ALU.add)

        # inverse via adjugate: M2 = 6x6 tiling of AAc (mod-3 access)
        M2 = solve_pool.tile([1, 36], F32, tag="M2", name="M2")
        for dr in (0, 3):
            for dc in (0, 3):
                nc.vector.tensor_copy(
                    out=_rd(M2[0:1, 6 * dr + dc: 6 * dr + dc + 1],
                            [[6, 3], [1, 3]]),
                    in_=AAc[0:1, :].rearrange("p (i j) -> p i j", j=3))
        t1 = solve_pool.tile([1, 9], F32, tag="t1", name="t1")
        t2 = solve_pool.tile([1, 9], F32, tag="t2", name="t2")
        nc.vector.tensor_mul(out=t1[0:1, :].rearrange("p (i j) -> p i j", j=3),
                             in0=_rd(M2[0:1, 7:8], [[6, 3], [1, 3]]),
                             in1=_rd(M2[0:1, 14:15], [[6, 3], [1, 3]]))
        nc.vector.tensor_mul(out=t2[0:1, :].rearrange("p (i j) -> p i j", j=3),
                             in0=_rd(M2[0:1, 8:9], [[6, 3], [1, 3]]),
                             in1=_rd(M2[0:1, 13:14], [[6, 3], [1, 3]]))
        cof = solve_pool.tile([1, 9], F32, tag="cof", name="cof")
        nc.vector.tensor_sub(out=cof[:], in0=t1[:], in1=t2[:])

        det = solve_pool.tile([1, 1], F32, tag="det", name="det")
        dscr = solve_pool.tile([1, 3], F32, tag="dscr", name="dscr")
        nc.vector.scalar_tensor_tensor(
            out=dscr[:], in0=AAc[0:1, 0:3], scalar=1.0, in1=cof[0:1, 0:3],
            op0=ALU.mult, op1=ALU.mult, accum_out=det[:])
        rdet = solve_pool.tile([1, 1], F32, tag="rdet", name="rdet")
        nc.vector.reciprocal(out=rdet[:], in_=det[:])

        inv9 = solve_pool.tile([1, 9], F32, tag="inv9", name="inv9")
        nc.vector.tensor_scalar_mul(
            out=inv9[0:1, :].rearrange("p (i j) -> p i j", j=3),
            in0=_rd(cof[0:1, 0:1], [[1, 3], [3, 3]]),  # cof^T
            scalar1=rdet[:])

        # x = BAc @ inv  (tmp27[i,k,j] = BAc[i,j]*inv[j,k], reduce j)
        tmp27 = solve_pool.tile([1, 27], F32, tag="tmp27", name="tmp27")
        nc.vector.tensor_mul(
            out=tmp27[0:1, :].rearrange("p (i k j) -> p i k j", k=3, j=3),
            in0=_rd(BAc[0:1, 0:1], [[3, 3], [0, 3], [1, 3]]),
            in1=_rd(inv9[0:1, 0:1], [[0, 3], [1, 3], [3, 3]]))
        x9 = solve_pool.tile([1, 9], F32, tag="x9", name="x9")
        nc.vector.reduce_sum(
            out=x9[0:1, :].rearrange("p (i k) -> p i k", k=3),
            in_=tmp27[0:1, :].rearrange("p (i k j) -> p i k j", k=3, j=3),
            axis=mybir.AxisListType.X)

        # d = Bm - x@Am
        tmp9 = solve_pool.tile([1, 9], F32, tag="tmp9", name="tmp9")
        nc.vector.tensor_mul(
            out=tmp9[0:1, :].rearrange("p (i j) -> p i j", j=3),
            in0=x9[0:1, :].rearrange("p (i j) -> p i j", j=3),
            in1=_rd(Am[0:1, 0:1], [[0, 3], [1, 3]]))
        xAm = solve_pool.tile([1, 3], F32, tag="xAm", name="xAm")
        nc.vector.reduce_sum(out=xAm[:], axis=mybir.AxisListType.X,
                             in_=tmp9[0:1, :].rearrange("p (i j) -> p i j", j=3))
        sol = solve_pool.tile([1, 12], F32, tag="sol", name="sol")
        nc.vector.tensor_copy(out=sol[0:1, 0:9], in_=x9[:])
        nc.vector.tensor_sub(out=sol[0:1, 9:12], in0=Bm[:], in1=xAm[:])

        # broadcast x,d to all partitions
        xb = solve_pool.tile([P, 12], F32, tag="xb", name="xb")
        nc.gpsimd.partition_broadcast(xb[:], sol[0:1, 0:12])

        # diag(x_ij) weight tiles
        dg = [[dg_pool.tile([P, P], F32, tag=f"dg{i}{j}", name=f"dg{i}{j}") for j in range(C)]
              for i in range(C)]
        for i in range(C):
            for j in range(C):
                nc.vector.tensor_scalar_mul(
                    out=dg[i][j][:], in0=eye[:],
                    scalar1=xb[:, 3 * i + j: 3 * i + j + 1])

        # ---------------- pass 2: out_i = sum_j x_ij A_j + d_i ----------------
        for g in range(NQ):
            for i in range(C):
                stage = stage_pool.tile([P, Q], F32, tag="stage", name="stage")
                pts = [ps_out.tile([P, MM], F32, tag="pt", name="pt") for _ in range(Q // MM)]
                for j in range(C):
                    for cc in range(Q // MM):
                        nc.tensor.matmul(
                            pts[cc][:], dg[i][j][:],
                            a_t[j][g][:, cc * MM:(cc + 1) * MM],
                            start=(j == 0), stop=(j == 2))
                for cc in range(Q // MM):
                    nc.scalar.add(out=stage[:, cc * MM:(cc + 1) * MM],
                                  in_=pts[cc][:], add=xb[:, 9 + i: 10 + i])
                nc.scalar.dma_start(out=outv[s, i][:, g * Q:(g + 1) * Q],
                                    in_=stage[:])


def build_nc() -> "bass.Bass":
    nc = bacc.Bacc("TRN2", target_bir_lowering=False)
    src = nc.dram_tensor("src", [B_CORE, C, H, W], F32, kind="ExternalInput")
    dst = nc.dram_tensor("dst", [B_CORE, C, H, W], F32, kind="ExternalInput")
    out = nc.dram_tensor("out", [B_CORE, C, H, W], F32, kind="ExternalOutput")
    with tile.TileContext(nc) as tc:
        _colornorm(tc, src[:], dst[:], out[:])
    nc.finalize()
    return nc


_NC = None


def _get_nc():
    global _NC
    if _NC is None:
        _NC = build_nc()
    return _NC


TRACE = False
LAST_RESULT = None  # BassKernelResults of the most recent run (for profiling)


def kernel(src, dst):
    from concourse.bass_utils import run_bass_kernel_spmd

    global LAST_RESULT
    src = np.ascontiguousarray(np.asarray(src, dtype=np.float32))
    dst = np.ascontiguousarray(np.asarray(dst, dtype=np.float32))
    assert src.shape == (NCORES * B_CORE, C, H, W), src.shape
    nc = _get_nc()
    in_maps = [
        {
            "src": np.ascontiguousarray(src[i * B_CORE:(i + 1) * B_CORE]),
            "dst": np.ascontiguousarray(dst[i * B_CORE:(i + 1) * B_CORE]),
        }
        for i in range(NCORES)
    ]
    res = run_bass_kernel_spmd(nc, in_maps, core_ids=list(range(NCORES)),
                               trace=TRACE)
    LAST_RESULT = res
    return np.concatenate([r["out"] for r in res.results], axis=0)
